# revision 28
# baseline (speedup 1.0000x reference)
"""Trainium2 Bass kernel for the GRAND attention block.

Shapes (hardcoded): B=16, C=1024, F=512, H=8, D=128, HD=1024.
Sharding: batch dim split across 8 cores (2 batches per core), weights
replicated; no collectives needed.

Math per batch (b):
  P_q = (x Wq + bq)/sqrt(D), P_k = x Wk + bk, P_v = x Wv + bv  [1024, 1024]
  The reference reshape [C, H*D] -> [H, C, D] (no permute) makes
  "head" g = proj_row // 128 and attention row c'' = 128*e + m where
  e = colblock, m = proj_row % 128.  Attention runs independently inside
  each group g of 1024 rows.  (bv must NOT be dropped: the bias index is
  (c2 mod 8)*128 + d after the permuted reshape, so it varies along the
  softmax axis and does not cancel against the zero row sums.)

    S^T tiles = K_e2 Q_e1^T  -> exp -> Z = E^T            [1024, 1024]
    r = column sums of Z (DVE tree + accumulating ones-matmuls)
    Z' = Z - diag(r)   (handles softmax denominator and the "- I" term)
    vals^T = (V_g^T Z') / r
    out^T += W0_g^T vals^T  (PSUM-accumulated over g), + bw0
  out^T is DMA-xbar transposed and stored with the unscrambling view.

Perf design (v3), driven by ntff engine profiles of v1/v2:
  - Host-staged bf16/fp8 operand layouts; Q/K projections as fp8
    DoubleRow matmuls (K folded 512 -> 2x256, weights prescaled by 64).
  - Engine economics (measured): a [128,1024] f32->bf16 drain costs
    1147ns on ACT but ~1500ns on DVE, so drains default to ACT; only
    HALF of the filler-phase QK drains go to DVE, to keep ACT's exp
    stream (the phase bottleneck) unclogged during batch-0 attention.
  - V drains: one full-width DVE add (+bv broadcast) per j instead of
    two halves.
  - Startup: wq8/wk8/wv halves split across the sync+gpsimd DMA queues;
    batch-0 units run Q0..7 then K0..7 then V0..7 so only wq8+xT8
    (1 MB on 2 queues) gates the first matmul.
  - Attention software-pipelined as in v1; batch 1's projection units
    drip into batch 0's attention as PE filler in Q/K/V round-robin
    order so the per-group ACT surcharge stays bounded.
"""

import functools
import math

import numpy as np
import ml_dtypes

import concourse.bass as bass
import concourse.bacc as bacc
import concourse.mybir as mybir
import concourse.tile as tile
from concourse.masks import make_identity
from concourse.bass_utils import run_bass_kernel_spmd

F32 = mybir.dt.float32
BF16 = mybir.dt.bfloat16
FP8 = mybir.dt.float8e4

N_CORES = 8
B_PER = 2  # batches per core
C = 1024
F = 512
H = 8
D = 128
HD = H * D
P = 128
KC = F // P  # 4 contraction chunks for the projections
INV_SQRT_D = 1.0 / math.sqrt(float(D))
W8SCALE = 64.0  # host pre-scale for fp8 Q/K weights (dodges e4m3 subnormals)
N_WARMUP = 16  # PE DVFS warm-up matmuls (N=512, ~215ns each) before data lands

Identity = mybir.ActivationFunctionType.Identity
Exp = mybir.ActivationFunctionType.Exp
DoubleRow = mybir.MatmulPerfMode.DoubleRow
Mult = mybir.AluOpType.mult
Add = mybir.AluOpType.add


def build_nc():
    nc = bacc.Bacc("TRN2", target_bir_lowering=False, debug=False)

    xT_d = nc.dram_tensor("xT", [B_PER, P, KC, C], BF16, kind="ExternalInput")
    xT8_d = nc.dram_tensor("xT8", [B_PER, P, KC, C], FP8, kind="ExternalInput")
    wq8_d = nc.dram_tensor("wq8", [P, KC, HD], FP8, kind="ExternalInput")
    wk8_d = nc.dram_tensor("wk8", [P, KC, HD], FP8, kind="ExternalInput")
    wv_d = nc.dram_tensor("wvc", [P, KC, HD], BF16, kind="ExternalInput")
    w0_d = nc.dram_tensor("w0c", [P, H, D], BF16, kind="ExternalInput")
    bqs_d = nc.dram_tensor("bqs", [P, H], F32, kind="ExternalInput")
    bks_d = nc.dram_tensor("bks", [P, H], F32, kind="ExternalInput")
    bv_d = nc.dram_tensor("bvh", [HD], BF16, kind="ExternalInput")
    bw0_d = nc.dram_tensor("bw0", [D], F32, kind="ExternalInput")
    out_d = nc.dram_tensor("out", [B_PER, C, D], BF16, kind="ExternalOutput")

    with tile.TileContext(nc) as tc:
        with (
            tc.tile_pool(name="const", bufs=1) as constp,
            tc.tile_pool(name="wts", bufs=1) as wtsp,
            tc.tile_pool(name="xt", bufs=2) as xtp,
            tc.tile_pool(name="projqk", bufs=2) as projp,
            tc.tile_pool(name="projv", bufs=2) as pvp,
            tc.tile_pool(name="z", bufs=2) as zp,
            tc.tile_pool(name="tree", bufs=2) as treep,
            tc.tile_pool(name="att", bufs=2) as attp,
            tc.tile_pool(name="outp", bufs=2) as outp,
            tc.tile_pool(name="ps2", bufs=2, space="PSUM") as ps2p,   # 2-bank tiles
            tc.tile_pool(name="ps1", bufs=2, space="PSUM") as ps1p,   # 1-bank tiles
            tc.tile_pool(name="psout", bufs=1, space="PSUM") as psoutp,  # 2 banks
        ):
            # ---- critical-path weight DMAs first: wq8 halves on the two
            # cheap queues (sync + gpsimd) so the first Q chain starts asap
            wq8 = wtsp.tile([P, KC, HD], FP8, name="wq8")
            # first Q matmul reads wq8[:, 0:2, 0:128] + xT8[:, 0:2, 0:512];
            # split the leading transfers so that dependency is ~128KB, not
            # 1MB, and the two queues deliver the halves concurrently
            nc.sync.dma_start(wq8[:, 0:2, 0:512], wq8_d[:, 0:2, 0:512])
            nc.gpsimd.dma_start(wq8[:, 0:2, 512:HD], wq8_d[:, 0:2, 512:HD])
            nc.sync.dma_start(wq8[:, 2:4, 0:512], wq8_d[:, 2:4, 0:512])
            nc.gpsimd.dma_start(wq8[:, 2:4, 512:HD], wq8_d[:, 2:4, 512:HD])
            wk8 = wtsp.tile([P, KC, HD], FP8, name="wk8")
            wv = wtsp.tile([P, KC, HD], BF16, name="wv")
            w0sb = constp.tile([P, H, D], BF16, name="w0sb")

            def emit_late_weights():
                # deferred until after batch 0's xT8 so the first Q chain
                # isn't queued behind not-yet-needed weights
                nc.sync.dma_start(wk8[:, 0:2, :], wk8_d[:, 0:2, :])
                nc.gpsimd.dma_start(wk8[:, 2:4, :], wk8_d[:, 2:4, :])
                nc.sync.dma_start(wv[:, 0:2, :], wv_d[:, 0:2, :])
                nc.gpsimd.dma_start(wv[:, 2:4, :], wv_d[:, 2:4, :])
                nc.gpsimd.dma_start(w0sb[:], w0_d[:])
                nc.gpsimd.dma_start(bvb[:], bv_d[None, :].to_broadcast([P, HD]))

            # ---- constants; the two memsets feeding the warm-up go first
            # so the warm-up matmuls can start right after the preamble ----
            ones = constp.tile([P, P], BF16, name="ones")
            nc.gpsimd.memset(ones, 1.0)
            bvb = constp.tile([P, HD], BF16, name="bvb")
            nc.gpsimd.memset(bvb, 0.0)

            # ---- PE DVFS warm-up: dense N=512 bf16 matmuls on constants,
            # sized to finish about when the first real operands land, so
            # real matmuls start at the high p-state instead of ramping ----
            ps_warm = ps2p.tile([P, C], F32, name="ps_warm", tag="ps2")
            for _ in range(N_WARMUP):
                nc.tensor.matmul(
                    ps_warm[:, 0:512],
                    lhsT=ones[:],
                    rhs=bvb[:, 0:512],
                    start=True,
                    stop=True,
                )

            ident = constp.tile([P, P], F32, name="ident")
            make_identity(nc, ident)
            ident4 = constp.tile([P, 4, P], BF16, name="ident4")
            for k in range(4):
                nc.vector.tensor_copy(out=ident4[:, k, :], in_=ident[:])

            bqss = constp.tile([P, H], F32, name="bqss")
            nc.scalar.dma_start(bqss[:], bqs_d[:])
            bksb = constp.tile([P, H], F32, name="bksb")
            nc.scalar.dma_start(bksb[:], bks_d[:])
            bw0sb = constp.tile([P, 1], F32, name="bw0sb")
            nc.scalar.dma_start(bw0sb[:], bw0_d[:, None])

            def emit_xt(b):
                # x^T tiles: straight per-chunk DMAs.  Batch 0 may use the
                # scalar queue (ACT is idle in the preamble); later batches
                # must keep off it so DMA-issue cost never blocks exps.
                xT8 = xtp.tile([P, KC, C], FP8, name="xT8", tag="xT8")
                for k in range(KC):
                    eng = nc.sync if k % 2 == 0 else nc.gpsimd
                    eng.dma_start(xT8[:, k, :], xT8_d[b, :, k, :])
                xT = xtp.tile([P, KC, C], BF16, name="xT", tag="xT")
                for k in range(KC):
                    if b == 0:
                        eng = nc.scalar if k % 2 == 0 else nc.sync
                    else:
                        eng = nc.sync if k % 2 == 0 else nc.gpsimd
                    eng.dma_start(xT[:, k, :], xT_d[b, :, k, :])
                return xT8, xT

            def make_proj(xT8, xT):
                """Allocate projection tiles; return them plus 24 chain
                emitters in Q0..7, K0..7, V0..7 order.  QK units accept
                drain_eng: "act" (default, cheaper per-op) or "vec"
                (used for half the filler drains to unclog ACT's exps)."""
                pqT = projp.tile([P, H, C], BF16, name="pqT", tag="pq")
                pkT = projp.tile([P, H, C], BF16, name="pkT", tag="pk")
                pv = pvp.tile([P, C // P, HD], BF16, name="pv", tag="pv")
                units = []

                def qk_unit(t, w8, bias, scale, dst, drain_eng="act"):
                    ps = ps2p.tile([P, C], F32, name="ps_qk", tag="ps2")
                    # a-outer order: the a=0 pass only needs x chunks 0-1,
                    # so the first units don't stall on the tail of the DMA
                    for a in range(2):
                        for s in range(2):
                            nc.tensor.matmul(
                                ps[:, 512 * s : 512 * (s + 1)],
                                lhsT=w8[:, 2 * a : 2 * a + 2, P * t : P * (t + 1)],
                                rhs=xT8[:, 2 * a : 2 * a + 2, 512 * s : 512 * (s + 1)],
                                start=(a == 0),
                                stop=(a == 1),
                                perf_mode=DoubleRow,
                                skip_group_check=True,
                            )
                    if drain_eng == "act":
                        nc.scalar.activation(
                            dst[:, t, :],
                            ps[:],
                            Identity,
                            bias=bias[:, t : t + 1],
                            scale=scale,
                        )
                    else:
                        nc.vector.tensor_scalar(
                            out=dst[:, t, :],
                            in0=ps[:],
                            scalar1=float(scale),
                            scalar2=bias[:, t : t + 1],
                            op0=Mult,
                            op1=Add,
                        )

                def v_unit(j, drain_eng="vec"):
                    ps = ps2p.tile([P, C], F32, name="ps_v", tag="ps2")
                    for s in range(2):
                        for k in range(KC):
                            nc.tensor.matmul(
                                ps[:, 512 * s : 512 * (s + 1)],
                                lhsT=xT[:, k, P * j : P * (j + 1)],
                                rhs=wv[:, k, 512 * s : 512 * (s + 1)],
                                start=(k == 0),
                                stop=(k == KC - 1),
                            )
                    nc.vector.tensor_add(out=pv[:, j, :], in0=ps[:], in1=bvb[:])

                for t in range(H):
                    units.append(
                        functools.partial(
                            qk_unit, t, wq8, bqss, INV_SQRT_D / W8SCALE, pqT
                        )
                    )
                for t in range(H):
                    units.append(
                        functools.partial(qk_unit, t, wk8, bksb, 1.0 / W8SCALE, pkT)
                    )
                for j in range(C // P):
                    units.append(functools.partial(v_unit, j))
                return (pqT, pkT, pv), units

            def emit_attention(batches, filler, pre=None):
                # attention over all batches' groups as ONE 16-entry group
                # pipeline.  Per group the emission stream interleaves the 8
                # score-pairs (which pace the ACT exp stream) with
                # quarter-chunks of the previous group's tail and with
                # filler units, so the in-order PE queue never ping-pongs
                # between an exp-paced scores phase (PE idle) and a dense
                # tail phase (ACT idle).  Running the two batches in one
                # stream lets batch 0's last tail + output drain overlap
                # batch 1's first score groups.  `pre` (batch 0's V units)
                # is threaded into group 0 one unit per score-step.
                zs = {}
                outaccs = {}

                def scores_step(bat, g, h2):
                    _, pqT, pkT, _ = bat
                    z, zs4 = zs[(bat[0], g)]
                    ps = ps2p.tile([P, C], F32, name="ps_s", tag="ps2")
                    for s in range(2):
                        nc.tensor.matmul(
                            ps[:, 512 * s : 512 * (s + 1)],
                            lhsT=pkT[:, h2, P * g : P * (g + 1)],
                            rhs=pqT[:, 4 * s : 4 * (s + 1), P * g : P * (g + 1)],
                            start=True,
                            stop=True,
                        )
                    nc.scalar.activation(z[:, h2, :], ps[:], Exp)
                    # incremental column-sum: pair-adds ride the exp stream;
                    # r chunks end up in zs4[:, 0] (=z0..z3), [:, 2], [:, 3]
                    if h2 % 2 == 1:
                        c = h2 // 2
                        nc.vector.tensor_add(
                            out=zs4[:, c, :],
                            in0=z[:, h2 - 1, :],
                            in1=z[:, h2, :],
                        )
                        if h2 == 3:
                            nc.vector.tensor_add(
                                out=zs4[:, 0, :],
                                in0=zs4[:, 0, :],
                                in1=zs4[:, 1, :],
                            )

                def tail_parts(bat, g):
                    """One group's reduction tail as 4 quarter emitters:
                    [r/diag s=0, r/diag s=1, vals+w0 s=0, vals+w0 s=1].
                    vals s-halves only touch/read their own column half of
                    z, so each part only depends on its own subs.  For the
                    batch's last group, the output drain chain rides on the
                    last quarter."""
                    b, _, _, pv = bat
                    outacc = outaccs[b]
                    z, zs4 = zs.pop((b, g))
                    vals = attp.tile([P, C], BF16, name="vals", tag="vals")
                    rcps = [None, None]

                    def rpart(s):
                        pr = ps1p.tile([P, 512], F32, name="pr", tag="ps1")
                        for i, a in enumerate((0, 2, 3)):
                            nc.tensor.matmul(
                                pr[:],
                                lhsT=ones[:],
                                rhs=zs4[:, a, 512 * s : 512 * (s + 1)],
                                start=(i == 0),
                                stop=(i == 2),
                            )
                        rcp = attp.tile([P, 512], F32, name="rcp", tag="rcp")
                        rcps[s] = rcp
                        nc.vector.reciprocal_approx_fast(rcp[:], pr[:])
                        dgm = attp.tile([P, 4, P], BF16, name="dgm", tag="dgm")
                        nc.vector.tensor_mul(
                            out=dgm[:],
                            in0=ident4[:],
                            in1=pr[:].rearrange("p (a j) -> p a j", j=P),
                        )
                        # s=0 subs on DVE (shortens the path to the first
                        # vals chain); s=1 subs on GpSimd in parallel
                        sub_eng = nc.vector if s == 0 else nc.gpsimd
                        for h2 in range(4 * s, 4 * (s + 1)):
                            sub_eng.tensor_sub(
                                out=z[:, h2, P * h2 : P * (h2 + 1)],
                                in0=z[:, h2, P * h2 : P * (h2 + 1)],
                                in1=dgm[:, h2 - 4 * s, :],
                            )

                    def vpart(s):
                        pvz = ps1p.tile([P, 512], F32, name="pvz", tag="ps1")
                        for h2 in range(H):
                            nc.tensor.matmul(
                                pvz[:],
                                lhsT=pv[:, g, P * h2 : P * (h2 + 1)],
                                rhs=z[:, h2, 512 * s : 512 * (s + 1)],
                                start=(h2 == 0),
                                stop=(h2 == 3 or h2 == H - 1),
                                skip_group_check=True,
                            )
                        nc.vector.tensor_mul(
                            out=vals[:, 512 * s : 512 * (s + 1)],
                            in0=pvz[:],
                            in1=rcps[s][:],
                        )
                        # w0 per half: the s=0 accumulator finishes one slot
                        # earlier, so the final drain/transpose overlaps the
                        # s=1 vals chain at the kernel tail
                        nc.tensor.matmul(
                            outacc[:, 512 * s : 512 * (s + 1)],
                            lhsT=w0sb[:, g, :],
                            rhs=vals[:, 512 * s : 512 * (s + 1)],
                            start=(g == 0),
                            stop=(g == H - 1),
                        )
                        if g == H - 1:
                            emit_out_drain(b, outacc, s)

                    return [
                        functools.partial(rpart, 0),
                        functools.partial(rpart, 1),
                        functools.partial(vpart, 0),
                        functools.partial(vpart, 1),
                    ]

                def emit_out_drain(b, outacc, s):
                    # drain + un-permute: xbar transpose + plain store, per
                    # 512-half on separate HWDGE queues; rides the last tail
                    # quarter so it overlaps the next batch's score groups
                    if s == 0:
                        outp_tiles[b] = (
                            outp.tile([P, C], BF16, name="outTb", tag="outTb"),
                            outp.tile([P, H, D], BF16, name="outTT", tag="outTT"),
                        )
                    outTb, outTT = outp_tiles[b]
                    out_r = out_d[b].rearrange("(cm e) d -> cm e d", e=H)
                    sl = slice(512 * s, 512 * (s + 1))
                    hs = slice(4 * s, 4 * (s + 1))
                    last = b == B_PER - 1
                    eng = nc.scalar if (last and s == 1) else nc.sync
                    nc.scalar.activation(
                        outTb[:, sl], outacc[:, sl], Identity, bias=bw0sb[:, 0:1]
                    )
                    eng.dma_start_transpose(outTT[:, hs, :], outTb[:, sl])
                    eng.dma_start(out_r[:, hs, :], outTT[:, hs, :])

                outp_tiles = {}

                # filler stream: the later batch's units in Q/K/V
                # round-robin; QK drains alternate act/vec so neither ACT
                # nor DVE saturates
                order = []
                if filler:
                    for t in range(H):
                        order.append((filler[t], "act" if t % 2 == 0 else "vec"))
                        order.append((filler[8 + t], "vec" if t % 2 == 0 else "act"))
                        order.append((filler[16 + t], None))
                fill = iter(order)

                def fill_one():
                    nxt = next(fill, None)
                    if nxt is not None:
                        u, eng = nxt
                        u(drain_eng=eng) if eng else u()
                        return True
                    return False

                pending = []  # tail quarters of the previous group
                for gi in range(len(batches) * H):
                    bat, g = batches[gi // H], gi % H
                    b = bat[0]
                    if g == 0:
                        outaccs[b] = psoutp.tile(
                            [P, C], F32, name="outacc", tag="outacc"
                        )
                    zs[(b, g)] = (
                        zp.tile([P, H, C], BF16, name="z", tag="z"),
                        treep.tile([P, 4, C], BF16, name="zs4", tag="zs4"),
                    )
                    for h2 in range(H):
                        if gi == 0 and pre and h2 < len(pre):
                            # thread batch 0's remaining V units into group
                            # 0: pure PE+DVE work (scores never wait on it),
                            # so the exp stream starts earlier than with a
                            # separate V phase
                            pre[h2]()
                        scores_step(bat, g, h2)
                        if h2 % 2 == 1 and pending:
                            pending.pop(0)()
                        elif h2 % 2 == 0:
                            fill_one()
                    pending = tail_parts(bat, g)
                for part in pending:
                    part()
                for _ in iter(lambda: fill_one(), False):
                    pass

            # ---- main flow: batch 0's Q+K units run standalone (gated only
            # on wq8/wk8/xT8 DMAs); its V units thread into group 0; both
            # batches' attention groups form one 16-group pipeline with
            # batch 1's projections dripped in as PE filler ----
            xt0 = emit_xt(0)
            emit_late_weights()
            proj0, units0 = make_proj(*xt0)
            for i, u in enumerate(units0[:16]):
                # alternate drains act/vec: DVE is idle here, and splitting
                # makes the QK phase PE-paced instead of ACT-paced
                u(drain_eng="act" if i % 2 == 0 else "vec")
            xt1 = emit_xt(1)
            proj1, units1 = make_proj(*xt1)
            # V0..V3 interleave with the tail of the K phase (wv/xT have
            # landed by then and the QK phase is drain-paced, so the PE has
            # slack); V4..V7 thread into group 0
            for i in range(4):
                units0[16 + i]()
            emit_attention(
                [(0, *proj0), (1, *proj1)],
                filler=units1,
                pre=units0[20:24],
            )

    return nc


_NC_CACHE = None


def _get_nc():
    global _NC_CACHE
    if _NC_CACHE is None:
        nc = build_nc()
        nc.compile()  # Bacc passes: move matmul waits to ldweights, alloc regs
        _NC_CACHE = nc
    return _NC_CACHE


def _install_ntff_shim():
    """The agent image's antenv lacks axon_hooks, so trn_boot's NTFF hook
    registration silently degrades. Recreate the module and register the
    ctypes-based hook so trace=True produces a profile."""
    import sys
    import types

    try:
        import antenv  # noqa: F401
        from antenv import axon_hooks  # noqa: F401

        return  # already present
    except ImportError:
        pass
    mod = types.ModuleType("antenv.axon_hooks")
    _state = {"hook": None}
    mod.set_axon_ntff_profile_hook = lambda h: _state.__setitem__("hook", h)
    mod.get_axon_ntff_profile_hook = lambda: _state["hook"]
    sys.modules["antenv.axon_hooks"] = mod
    import antenv

    antenv.axon_hooks = mod
    try:
        from trn_agent_boot.trn_boot import _ntff_profile_via_ctypes

        hook = _ntff_profile_via_ctypes("/opt/axon/libaxon_pjrt.so")
        if hook is not None:
            mod.set_axon_ntff_profile_hook(hook)
    except Exception as e:  # degrade to no tracing
        print(f"ntff shim failed: {e}")


def _host_stage(inputs):
    """Cast/layout all operands on the host so the device DMAs bf16/fp8."""
    f32 = np.float32
    bf16 = ml_dtypes.bfloat16
    fp8 = ml_dtypes.float8_e4m3fn

    Wq = np.asarray(inputs["Wq"], f32)
    Wk = np.asarray(inputs["Wk"], f32)
    Wv = np.asarray(inputs["Wv"], f32)

    def chunk(w):  # [F, HD] -> [P, KC, HD]  (w[128k+p, hd] -> [p, k, hd])
        return np.ascontiguousarray(w.reshape(KC, P, HD).transpose(1, 0, 2))

    weights = {
        "wq8": (chunk(Wq) * W8SCALE).astype(fp8),
        "wk8": (chunk(Wk) * W8SCALE).astype(fp8),
        "wvc": chunk(Wv).astype(bf16),
        "w0c": np.ascontiguousarray(
            np.asarray(inputs["Ww0"], f32).reshape(H, P, D).transpose(1, 0, 2)
        ).astype(bf16),
        "bqs": np.ascontiguousarray(
            (np.asarray(inputs["bq"], f32) * INV_SQRT_D).reshape(H, P).T
        ),
        "bks": np.ascontiguousarray(np.asarray(inputs["bk"], f32).reshape(H, P).T),
        "bvh": np.asarray(inputs["bv"], f32).astype(bf16),
        "bw0": np.asarray(inputs["bw0"], f32),
    }

    x = np.asarray(inputs["x"], f32)  # [B, C, F]
    # xT[b, p, k, c] = x[b, c, 128k + p]
    xT = np.ascontiguousarray(
        x.transpose(0, 2, 1).reshape(x.shape[0], KC, P, C).transpose(0, 2, 1, 3)
    )
    return weights, xT.astype(bf16), xT.astype(fp8)


def kernel_with_results(trace=False, **inputs):
    if trace:
        _install_ntff_shim()
    nc = _get_nc()
    weights, xT, xT8 = _host_stage(inputs)
    in_maps = []
    for i in range(N_CORES):
        m = {
            "xT": np.ascontiguousarray(xT[B_PER * i : B_PER * (i + 1)]),
            "xT8": np.ascontiguousarray(xT8[B_PER * i : B_PER * (i + 1)]),
        }
        m.update(weights)
        in_maps.append(m)
    res = run_bass_kernel_spmd(nc, in_maps, list(range(N_CORES)), trace=trace)
    out = np.concatenate(
        [res.results[i]["out"].astype(np.float32) for i in range(N_CORES)], axis=0
    )
    return out, res


def kernel(**inputs):
    out, _ = kernel_with_results(trace=False, **inputs)
    return out
